# revision 6
# baseline (speedup 1.0000x reference)
"""Trainium2 Bass kernel for MultiInnerProductDecoder (DistMult edge scoring).

score_e = sigmoid( sum_d z[src_e, d] * z[dst_e, d] * weight[type_e, d] )

Sharding: edges split evenly across 8 NeuronCores (data parallel over E);
z and weight replicated on every core. Each core gathers z[src], z[dst],
weight[et] rows via SWDGE indirect DMA (128 rows per instruction — the only
gather primitive available on this image), combines on DVE, sigmoid on ACT.
"""

import numpy as np

# Problem constants (hardcoded per harness contract — no spec.json reads).
N_DRUGS = 100000
NUM_ET = 1000
IN_DIM = 128  # D
N_EDGES = 2000000
N_CORES = 8
E_PER_CORE = N_EDGES // N_CORES  # 250000

P = 128            # SBUF partitions
K = 32             # gather columns per superbatch
BATCH = P * K      # 4096 edges per superbatch
NB = -(-E_PER_CORE // BATCH)   # 62 superbatches
E_PAD = NB * BATCH             # 253952 padded edges per core

_cache = {}


def _build_nc():
    import concourse.bass as bass
    import concourse.tile as tile
    from concourse import bacc, mybir

    f32 = mybir.dt.float32
    i32 = mybir.dt.int32
    D = IN_DIM

    # Bacc (not plain Bass): its compile() pipeline runs
    # generate_event_semaphores, which splits multi-sem waits into
    # EventSemaphore instructions (TRN2 allows at most 1 wait per inst).
    nc = bacc.Bacc(None)
    z_ext = nc.declare_dram_parameter("z", [N_DRUGS, D], f32, isOutput=False)
    w_ext = nc.declare_dram_parameter("weight", [NUM_ET, D], f32, isOutput=False)
    src_ext = nc.declare_dram_parameter("src", [E_PAD], i32, isOutput=False)
    dst_ext = nc.declare_dram_parameter("dst", [E_PAD], i32, isOutput=False)
    et_ext = nc.declare_dram_parameter("et", [E_PAD], i32, isOutput=False)
    out_ext = nc.declare_dram_parameter("out", [E_PAD], f32, isOutput=True)

    # Edge order convention: edge (b, p, k) lives at flat position
    # b*BATCH + p*K + k in every per-core array (src/dst/et/out).
    srcv = src_ext[:].rearrange("(b p k) -> p b k", p=P, k=K)
    dstv = dst_ext[:].rearrange("(b p k) -> p b k", p=P, k=K)
    etv = et_ext[:].rearrange("(b p k) -> p b k", p=P, k=K)
    outv = out_ext[:].rearrange("(b p k) -> b p k", p=P, k=K)

    def gather128(dst_slice, table, offs):
        # One 128-row gather: row offs[p] of `table` -> dst_slice[p, :].
        nc.gpsimd.indirect_dma_start(
            out=dst_slice,
            out_offset=None,
            in_=table,
            in_offset=bass.IndirectOffsetOnAxis(ap=offs, axis=0),
        )

    with tile.TileContext(nc) as tc:
        with (
            tc.tile_pool(name="idx", bufs=1) as idxp,
            tc.tile_pool(name="gsrc", bufs=2) as psrc,
            tc.tile_pool(name="gdst", bufs=2) as pdst,
            tc.tile_pool(name="gw", bufs=2) as pw,
            tc.tile_pool(name="score", bufs=3) as scp,
        ):
            # Preload ALL edge indices once into persistent SBUF tiles
            # ([128, NB*K] i32 each = ~8KB/partition).
            src_all = idxp.tile([P, NB, K], i32, tag="src_all")
            nc.sync.dma_start(out=src_all[:], in_=srcv)
            dst_all = idxp.tile([P, NB, K], i32, tag="dst_all")
            nc.sync.dma_start(out=dst_all[:], in_=dstv)
            et_all = idxp.tile([P, NB, K], i32, tag="et_all")
            nc.sync.dma_start(out=et_all[:], in_=etv)

            for b in range(NB):
                zs = psrc.tile([P, K * D], f32, tag="zs")
                zd = pdst.tile([P, K * D], f32, tag="zd")
                wt = pw.tile([P, K * D], f32, tag="wt")
                for k in range(K):
                    sl = slice(k * D, (k + 1) * D)
                    gather128(zs[:, sl], z_ext[:, :], src_all[:, b, k : k + 1])
                    gather128(zd[:, sl], z_ext[:, :], dst_all[:, b, k : k + 1])
                    gather128(wt[:, sl], w_ext[:, :], et_all[:, b, k : k + 1])

                nc.vector.tensor_mul(zs[:], zs[:], zd[:])
                nc.vector.tensor_mul(zs[:], zs[:], wt[:])

                sc = scp.tile([P, K], f32, tag="sc")
                nc.vector.reduce_sum(
                    out=sc[:],
                    in_=zs[:].rearrange("p (k d) -> p k d", d=D),
                    axis=mybir.AxisListType.X,
                )
                sg = scp.tile([P, K], f32, tag="sg")
                nc.scalar.activation(
                    out=sg[:], in_=sc[:],
                    func=mybir.ActivationFunctionType.Sigmoid,
                )
                nc.sync.dma_start(out=outv[b], in_=sg[:])

    nc.compile()
    return nc


def _shard_inputs(z, edge_index, edge_type, weight):
    z = np.ascontiguousarray(np.asarray(z, dtype=np.float32))
    weight = np.ascontiguousarray(np.asarray(weight, dtype=np.float32))
    edge_index = np.asarray(edge_index)
    edge_type = np.asarray(edge_type)

    in_maps = []
    for c in range(N_CORES):
        lo, hi = c * E_PER_CORE, (c + 1) * E_PER_CORE
        src = np.zeros(E_PAD, np.int32)
        dst = np.zeros(E_PAD, np.int32)
        et = np.zeros(E_PAD, np.int32)
        src[:E_PER_CORE] = edge_index[0, lo:hi]
        dst[:E_PER_CORE] = edge_index[1, lo:hi]
        et[:E_PER_CORE] = edge_type[lo:hi]
        in_maps.append(
            {"z": z, "weight": weight, "src": src, "dst": dst, "et": et}
        )
    return in_maps


def run(z, edge_index, edge_type, weight, trace=False, **trace_kw):
    """Returns (output [N_EDGES] f32, BassKernelResults)."""
    from concourse.bass_utils import run_bass_kernel_spmd

    if "nc" not in _cache:
        _cache["nc"] = _build_nc()
    nc = _cache["nc"]

    in_maps = _shard_inputs(z, edge_index, edge_type, weight)
    res = run_bass_kernel_spmd(
        nc, in_maps, core_ids=list(range(N_CORES)), trace=trace, **trace_kw
    )
    out = np.concatenate(
        [res.results[c]["out"][:E_PER_CORE] for c in range(N_CORES)]
    )
    return out, res


def kernel(z, edge_index, edge_type, weight):
    out, _ = run(z, edge_index, edge_type, weight)
    return out


# revision 7
# speedup vs baseline: 11.3258x; 11.3258x over previous
"""Trainium2 Bass kernel for MultiInnerProductDecoder (DistMult edge scoring).

score_e = sigmoid( sum_d z[src_e, d] * z[dst_e, d] * weight[type_e, d] )

Sharding: edges split evenly across 8 NeuronCores (data parallel over E);
z and weight replicated on every core.

Per-core strategy: z[src] and z[dst] rows are fetched with SWDGE indirect
DMA (128 rows x 512B per instruction — the only gather primitive available
on this image; ~1.2us of Pool-engine descriptor generation each). To avoid
a third per-edge gather for weight[et], the host sorts each core's edges by
edge type and pads every type run to a multiple of 32, so each partition-row
(32 edges) of a superbatch has a single type: one 128-row weight gather then
serves a whole 4096-edge superbatch, with the weight row broadcast along the
free dim in the multiply. The host un-permutes the returned scores.
"""

import numpy as np

# Problem constants (hardcoded per harness contract — no spec.json reads).
N_DRUGS = 100000
NUM_ET = 1000
IN_DIM = 128  # D
N_EDGES = 2000000
N_CORES = 8
E_PER_CORE = N_EDGES // N_CORES  # 250000

P = 128            # SBUF partitions
K = 32             # edges per partition-row per superbatch
BATCH = P * K      # 4096 edges per superbatch

_cache = {}


def _build_nc(nb):
    import concourse.bass as bass
    import concourse.tile as tile
    from concourse import bacc, mybir

    f32 = mybir.dt.float32
    i32 = mybir.dt.int32
    D = IN_DIM
    slots = nb * BATCH

    # Bacc (not plain Bass): its compile() pipeline runs
    # generate_event_semaphores, which splits multi-sem waits into
    # EventSemaphore instructions (TRN2 allows at most 1 wait per inst).
    nc = bacc.Bacc(None)
    z_ext = nc.declare_dram_parameter("z", [N_DRUGS, D], f32, isOutput=False)
    w_ext = nc.declare_dram_parameter("weight", [NUM_ET, D], f32, isOutput=False)
    src_ext = nc.declare_dram_parameter("src", [slots], i32, isOutput=False)
    dst_ext = nc.declare_dram_parameter("dst", [slots], i32, isOutput=False)
    rty_ext = nc.declare_dram_parameter("rty", [nb * P], i32, isOutput=False)
    out_ext = nc.declare_dram_parameter("out", [slots], f32, isOutput=True)

    # Slot order convention: slot (b, p, k) is flat position b*BATCH + p*K + k
    # in src/dst/out; row (b, p) of rty holds that row's edge type.
    srcv = src_ext[:].rearrange("(b p k) -> p b k", p=P, k=K)
    dstv = dst_ext[:].rearrange("(b p k) -> p b k", p=P, k=K)
    rtyv = rty_ext[:].rearrange("(b p) -> p b", p=P)
    outv = out_ext[:].rearrange("(b p k) -> b p k", p=P, k=K)

    def gather128(dst_slice, table, offs):
        # One 128-row gather: row offs[p] of `table` -> dst_slice[p, :].
        nc.gpsimd.indirect_dma_start(
            out=dst_slice,
            out_offset=None,
            in_=table,
            in_offset=bass.IndirectOffsetOnAxis(ap=offs, axis=0),
        )

    with tile.TileContext(nc) as tc:
        with (
            tc.tile_pool(name="idx", bufs=1) as idxp,
            tc.tile_pool(name="gsrc", bufs=3) as psrc,
            tc.tile_pool(name="gdst", bufs=3) as pdst,
            tc.tile_pool(name="gw", bufs=3) as pw,
            tc.tile_pool(name="score", bufs=3) as scp,
        ):
            # Preload ALL indices once into persistent SBUF tiles.
            src_all = idxp.tile([P, nb, K], i32, tag="src_all")
            nc.sync.dma_start(out=src_all[:], in_=srcv)
            dst_all = idxp.tile([P, nb, K], i32, tag="dst_all")
            nc.sync.dma_start(out=dst_all[:], in_=dstv)
            rty_all = idxp.tile([P, nb], i32, tag="rty_all")
            nc.sync.dma_start(out=rty_all[:], in_=rtyv)

            for b in range(nb):
                wt = pw.tile([P, D], f32, tag="wt")
                gather128(wt[:], w_ext[:, :], rty_all[:, b : b + 1])
                zs = psrc.tile([P, K * D], f32, tag="zs")
                zd = pdst.tile([P, K * D], f32, tag="zd")
                for k in range(K):
                    sl = slice(k * D, (k + 1) * D)
                    gather128(zs[:, sl], z_ext[:, :], src_all[:, b, k : k + 1])
                    gather128(zd[:, sl], z_ext[:, :], dst_all[:, b, k : k + 1])

                nc.vector.tensor_mul(zs[:], zs[:], zd[:])
                # weight row is per partition-row: broadcast along k.
                nc.vector.tensor_mul(
                    zs[:].rearrange("p (k d) -> p k d", d=D),
                    zs[:].rearrange("p (k d) -> p k d", d=D),
                    wt[:].rearrange("p (k d) -> p k d", k=1).to_broadcast([P, K, D]),
                )

                sc = scp.tile([P, K], f32, tag="sc")
                nc.vector.reduce_sum(
                    out=sc[:],
                    in_=zs[:].rearrange("p (k d) -> p k d", d=D),
                    axis=mybir.AxisListType.X,
                )
                sg = scp.tile([P, K], f32, tag="sg")
                nc.scalar.activation(
                    out=sg[:], in_=sc[:],
                    func=mybir.ActivationFunctionType.Sigmoid,
                )
                nc.sync.dma_start(out=outv[b], in_=sg[:])

    nc.compile()
    return nc


def _shard_inputs(z, edge_index, edge_type, weight):
    z = np.ascontiguousarray(np.asarray(z, dtype=np.float32))
    weight = np.ascontiguousarray(np.asarray(weight, dtype=np.float32))
    edge_index = np.asarray(edge_index)
    edge_type = np.asarray(edge_type)

    cores = []
    nb = 0
    for c in range(N_CORES):
        lo, hi = c * E_PER_CORE, (c + 1) * E_PER_CORE
        src = np.asarray(edge_index[0, lo:hi], dtype=np.int64)
        dst = np.asarray(edge_index[1, lo:hi], dtype=np.int64)
        et = np.asarray(edge_type[lo:hi], dtype=np.int64)

        order = np.argsort(et, kind="stable")
        counts = np.bincount(et, minlength=NUM_ET)
        padded = counts + (-counts) % K
        used = int(padded.sum())
        cores.append((src, dst, et, order, counts, padded, used))
        nb = max(nb, -(-used // BATCH))

    slots = nb * BATCH
    in_maps, unscramble = [], []
    for src, dst, et, order, counts, padded, used in cores:
        csum = np.cumsum(padded) - padded       # padded start per type
        jj = np.arange(E_PER_CORE) - np.repeat(
            np.cumsum(counts) - counts, counts
        )
        q = np.repeat(csum, counts) + jj        # slot of sorted edge i

        src_s = np.zeros(slots, np.int32)
        dst_s = np.zeros(slots, np.int32)
        src_s[q] = src[order]
        dst_s[q] = dst[order]
        rowt = np.zeros(nb * P, np.int32)
        filled = np.repeat(np.arange(NUM_ET), padded // K)
        rowt[: filled.shape[0]] = filled

        in_maps.append({
            "z": z, "weight": weight,
            "src": src_s, "dst": dst_s, "rty": rowt,
        })
        unscramble.append((order, q))
    return nb, in_maps, unscramble


def run(z, edge_index, edge_type, weight, trace=False, **trace_kw):
    """Returns (output [N_EDGES] f32, BassKernelResults)."""
    from concourse.bass_utils import run_bass_kernel_spmd

    nb, in_maps, unscramble = _shard_inputs(z, edge_index, edge_type, weight)
    if nb not in _cache:
        _cache[nb] = _build_nc(nb)
    nc = _cache[nb]

    res = run_bass_kernel_spmd(
        nc, in_maps, core_ids=list(range(N_CORES)), trace=trace, **trace_kw
    )
    out = np.empty(N_EDGES, np.float32)
    for c in range(N_CORES):
        order, q = unscramble[c]
        piece = np.empty(E_PER_CORE, np.float32)
        piece[order] = res.results[c]["out"][q]
        out[c * E_PER_CORE : (c + 1) * E_PER_CORE] = piece
    return out, res


def kernel(z, edge_index, edge_type, weight):
    out, _ = run(z, edge_index, edge_type, weight)
    return out
